# revision 37
# baseline (speedup 1.0000x reference)
"""MemristorDense Trainium2 kernel (8 NeuronCores, SPMD tensor-parallel).

Per core (128 interleaved columns host-reordered to [64 pos | 64 neg]):
  y[b,o] = I[b,o] - I[b,o+64],
  I[b,j] = sum_i (0.5|w|+cmw) * r^E,   r = 2*inputs, E = log2 n,
  cmw = 0.5*max|w_slice|/99  (local max: only perturbs the ~1% G_MIN
  bias; ~3e-5 rel impact).
The bias input row (i=1024, input 1) rides along as chunk-8 data:
x chunk 8 is [1, floor...] so pad rows give w=0 * c0~0 = 0 exactly.
Series around mu: r^E = e^{mu L} sum_k (L d)^k / k!,  L = ln r, d = E-mu.
Engine mapping:
  C0  = e^{mu L - ln2} f16        (0.5 folded into the Exp bias)
  Lp  = max(L/ln2, -144) bf16     (one fused tensor_scalar)
  dl  = ln(n * e^{-mu ln2}) bf16  (= ln n - mu ln2, via the Ln scale)
  ab  = max(w, -w) f16;  w0f = ab + 2*cmw f16
  C_k = C_{k-1} * (Lp/k),  W_k = W_{k-1} * dl    (bf16 TensorTensor)
  I   = C0^T@ab (f16 mm; the missing cmw part of k=0 is column-constant
        and cancels exactly in the pos-neg diff) + sum_k C_k^T@W_k.
Total err ~3.7e-3 (fp16 inputs + bf16 chains) vs the 2e-2 gate.
Inputs DMA as fp16: x blocked [P,9,B] (host-floored at 6.2e-5 so Ln
never sees 0/denormals) and split across both HWDGE queues into two
tiles so Ln starts as soon as the first half lands; (w,n) blocked
[P,2,9,JC].
"""

from contextlib import ExitStack

import numpy as np

import concourse.bass as bass
import concourse.bass_isa as bass_isa
import concourse.tile as tile
from concourse import bacc
from concourse import mybir
from concourse import bass_utils

P = 128
B = 128
N_IN = 1024
N_OUT = 512
NCH = 9                 # i-chunks of 128 (1025 rows padded to 1152)
IPAD = NCH * P
JC = 128                # columns per core
NO = JC // 2            # outputs per core
NCORES = 8
K_TERMS = 3             # series terms k = 0..3

MU = 1.58
LN2 = float(np.log(2.0))
INV_LN2 = 1.0 / LN2
MULN2 = MU * LN2
S_N = float(np.exp(-MULN2))   # Ln scale: ln(n*S_N) = ln n - mu ln2
CB2 = 1.0 / 99.0              # cmw2 = mw/99 (2x cmw; 0.5 lives in C0)
LP_CLAMP = -144.0
C0_BIAS = float(0.5 * 2.0 ** MU)    # bias-row C0 = 0.5*2^mu
X_FLOOR = 6.2e-5              # fp16 min normal; applied in host cast

F32 = mybir.dt.float32
F16 = mybir.dt.float16
BF16 = mybir.dt.bfloat16
AF = mybir.ActivationFunctionType
ALU = mybir.AluOpType

_NC_CACHE = None


def _kernel_body(ctx, tc, xt, wn, y):
    nc = tc.nc

    pool = ctx.enter_context(tc.tile_pool(name="main", bufs=2))
    cpool = ctx.enter_context(tc.tile_pool(name="cpool", bufs=K_TERMS))
    wpool = ctx.enter_context(tc.tile_pool(name="wpool", bufs=K_TERMS))
    psum = ctx.enter_context(tc.tile_pool(name="psum", bufs=2, space="PSUM"))

    # ---- loads on qSP (pre-blocked [p, ...]; one contiguous run per
    # partition); x first: it heads the Ln->Exp critical chain. ----
    xta = pool.tile([P, 5, B], F16, tag="xta")
    nc.sync.dma_start(xta[:], xt.ap()[:, 0:5])
    xtb = pool.tile([P, 3, B], F16, tag="xtb")
    nc.scalar.dma_start(xtb[:], xt.ap()[:, 5:8])
    wnt = pool.tile([P, 2, NCH, JC], F16, tag="wn")
    nc.scalar.dma_start(wnt[:, 1], wn.ap()[:, 1])
    nc.sync.dma_start(wnt[:, 0], wn.ap()[:, 0])

    # ---- ACT: L = ln(2x); dl = ln n - mu ln2 (via scale); C0 = 0.5 e^{mu L}.
    # Both Ln's precede Exp so only two act-table loads happen. ----
    eb = pool.tile([P, 1], F32, tag="eb")
    nc.any.memset(eb[:], -LN2)
    lt = pool.tile([P, 8, B], F32, tag="lt")
    nc.scalar.activation(lt[:, 0:5], xta[:], AF.Ln, bias=0.0, scale=2.0)
    nc.scalar.activation(lt[:, 5:8], xtb[:], AF.Ln, bias=0.0, scale=2.0)
    dl = pool.tile([P, NCH, JC], BF16, tag="dl")
    nc.scalar.activation(dl[:], wnt[:, 1], AF.Ln, bias=0.0, scale=S_N)
    # c0 in two tiles at the Ln seam; chunk 8 is constant (bias row
    # C0=0.5*2^mu at p=0, exact 0 on pad rows) via memsets
    c0a = pool.tile([P, 5, B], F16, tag="c0a")
    nc.scalar.activation(c0a[:], lt[:, 0:5], AF.Exp, bias=eb[:], scale=MU)
    c0b = pool.tile([P, 4, B], F16, tag="c0b")
    nc.scalar.activation(c0b[:, 0:3], lt[:, 5:8], AF.Exp, bias=eb[:], scale=MU)
    nc.gpsimd.memset(c0b[:, 3, :], 0.0)
    nc.gpsimd.memset(c0b[0:1, 3, :], C0_BIAS)

    # ---- ab = |w| = max(w, -w) f16 (k=0 rhs, cmw-free) ----
    ngw = pool.tile([P, NCH, JC], F16, tag="ngw")
    nc.vector.tensor_scalar_mul(ngw[:], wnt[:, 0], -1.0)
    ab = pool.tile([P, NCH, JC], F16, tag="ab")
    nc.vector.tensor_tensor(ab[:], wnt[:, 0], ngw[:], op=ALU.max)

    # ---- local max |w| (bias row rides in chunk 8) -> cmw2 = mw/99 ----
    rm = pool.tile([P, 1], F32, tag="rm")
    nc.vector.tensor_reduce(
        rm[:], wnt[:, 0], axis=mybir.AxisListType.XY, op=ALU.max,
        apply_absolute_value=True,
    )
    mwall = pool.tile([P, 1], F32, tag="mwall")
    nc.gpsimd.partition_all_reduce(
        mwall[:], rm[:], channels=P, reduce_op=bass_isa.ReduceOp.max
    )
    cmw = pool.tile([P, 1], F32, tag="cmw")
    nc.vector.tensor_scalar_mul(cmw[:], mwall[:], CB2)

    # ---- Lp = clamp(L/ln2) bf16 + prescaled Lp/k for the C-chain ----
    lp = pool.tile([P, NCH, B], BF16, tag="lp")
    nc.vector.tensor_scalar(lp[:, 0:8], lt[:], INV_LN2, LP_CLAMP, op0=ALU.mult, op1=ALU.max)
    nc.gpsimd.memset(lp[:, 8, :], 0.0)
    nc.gpsimd.memset(lp[0:1, 8, :], 1.0)
    lpk = {1: lp}
    for k in range(2, K_TERMS + 1):
        t = pool.tile([P, NCH, B], BF16, tag=f"lp{k}")
        nc.vector.tensor_scalar_mul(t[:], lp[:], 1.0 / k)
        lpk[k] = t

    w0f = pool.tile([P, NCH, JC], F16, tag="w0f")
    nc.vector.tensor_scalar(w0f[:], ab[:], 1.0, cmw[:], op0=ALU.mult, op1=ALU.add)

    # ---- PSUM accumulation: k=0 (f16), k=1..3 (bf16 chains).
    # W-link emitted before C-link: the W side is ready earlier. ----
    ps = psum.tile([B, JC], F32, tag="acc")
    for c in range(NCH):
        lhsT = c0a[:, c, :] if c < 5 else c0b[:, c - 5, :]
        nc.tensor.matmul(
            ps[:], lhsT=lhsT, rhs=ab[:, c, :], start=(c == 0), stop=False,
        )
    ck, wk = None, w0f
    for k in range(1, K_TERMS + 1):
        wnew = wpool.tile([P, NCH, JC], BF16, tag="wk")
        nc.vector.tensor_mul(wnew[:], wk[:], dl[:])
        cn = cpool.tile([P, NCH, B], BF16, tag="ck")
        if k == 1:
            # C1 split at the Exp seam so the terminal link is a half-op
            nc.vector.tensor_mul(cn[:, 0:5], c0a[:], lpk[1][:, 0:5])
            nc.vector.tensor_mul(cn[:, 5:NCH], c0b[:], lpk[1][:, 5:NCH])
        else:
            nc.vector.tensor_mul(cn[:], ck[:], lpk[k][:])
        ck, wk = cn, wnew
        for c in range(NCH):
            nc.tensor.matmul(
                ps[:], lhsT=ck[:, c, :], rhs=wk[:, c, :],
                start=False, stop=(k == K_TERMS and c == NCH - 1),
            )

    # ---- y = pos block - neg block (host re-ordered columns) ----
    yp = pool.tile([B, NO], F32, tag="yp")
    nc.scalar.activation(yp[:], ps[:, 0:NO], AF.Copy, bias=0.0, scale=1.0)
    yt = pool.tile([B, NO], F32, tag="yt")
    nc.vector.tensor_sub(yt[:], yp[:], ps[:, NO:JC])
    nc.scalar.dma_start(y.ap(), yt[:])


def build_nc(repeat=1):
    nc = bacc.Bacc(
        "TRN2", target_bir_lowering=False, debug=False, num_devices=NCORES
    )
    xt = nc.dram_tensor("xt", [P, NCH, B], F16, kind="ExternalInput")
    wn = nc.dram_tensor("wn", [P, 2, NCH, JC], F16, kind="ExternalInput")
    y = nc.dram_tensor("y", [B, NO], F32, kind="ExternalOutput")
    with tile.TileContext(nc) as tc:
        with ExitStack() as ctx:
            if repeat == 1:
                _kernel_body(ctx, tc, xt, wn, y)
            else:
                with tc.For_i(0, repeat, 1):
                    _kernel_body(ctx, tc, xt, wn, y)
    nc.compile()
    return nc


def _block(a):
    """[NCH*P, W] row-major -> [P, NCH, W] partition-major contiguous."""
    n, w = a.shape
    return a.reshape(n // P, P, w).transpose(1, 0, 2)


def make_in_maps(x, w_pos, w_neg, b_pos, b_neg, n_devices):
    comb = np.zeros((IPAD, 2 * N_OUT), np.float32)
    comb[:N_IN, 0::2] = w_pos
    comb[:N_IN, 1::2] = w_neg
    comb[N_IN, 0::2] = b_pos
    comb[N_IN, 1::2] = b_neg
    nfull = np.full((IPAD, 2 * N_OUT), 2.0, np.float32)
    nfull[:N_IN + 1] = np.asarray(n_devices, np.float32)
    # inputs with the bias-1 row at i=1024; pad rows floored (w=0 there)
    xfull = np.full((IPAD, B), X_FLOOR, np.float32)
    xfull[:N_IN] = np.asarray(x, np.float32).T
    xfull[N_IN] = 1.0
    xq = np.maximum(xfull.astype(np.float16), np.float16(X_FLOOR))
    xb = np.ascontiguousarray(_block(xq))            # [P, NCH, B]
    # within-core column order: 64 pos then 64 neg
    perm = np.r_[np.arange(0, JC, 2), np.arange(1, JC, 2)]
    in_maps = []
    for core in range(NCORES):
        js = slice(JC * core, JC * (core + 1))
        wc = comb[:, js][:, perm]
        ncr = nfull[:, js][:, perm]
        wnb = np.stack([_block(wc), _block(ncr)], axis=1).astype(np.float16)
        in_maps.append({"xt": xb, "wn": np.ascontiguousarray(wnb)})
    return in_maps


def gather(results):
    return np.concatenate(
        [np.asarray(results[c]["y"], np.float32) for c in range(NCORES)], axis=1
    )


def _get_nc():
    global _NC_CACHE
    if _NC_CACHE is None:
        _NC_CACHE = build_nc()
    return _NC_CACHE


def kernel(x, w_pos, w_neg, b_pos, b_neg, n_devices):
    in_maps = make_in_maps(x, w_pos, w_neg, b_pos, b_neg, n_devices)
    res = bass_utils.run_bass_kernel_spmd(
        _get_nc(), in_maps, core_ids=list(range(NCORES))
    )
    return gather(res.results)


# revision 38
# speedup vs baseline: 1.2192x; 1.2192x over previous
"""MemristorDense Trainium2 kernel (8 NeuronCores, SPMD tensor-parallel).

Per core (128 interleaved columns host-reordered to [64 pos | 64 neg]):
  y[b,o] = I[b,o] - I[b,o+64],
  I[b,j] = sum_i (0.5|w|+cmw) * r^E,   r = 2*inputs, E = log2 n,
  cmw = 0.5*max|w_slice|/99  (local max: only perturbs the ~1% G_MIN
  bias; ~3e-5 rel impact).
The bias input row (i=1024, input 1) rides along as chunk-8 data:
x chunk 8 is [1, floor...] so pad rows give w=0 * c0~0 = 0 exactly.
Series around mu: r^E = e^{mu L} sum_k (L d)^k / k!,  L = ln r, d = E-mu.
Engine mapping:
  C0  = e^{mu L - ln2} f16        (0.5 folded into the Exp bias)
  Lp  = max(L/ln2, -144) bf16     (one fused tensor_scalar)
  dl  = ln(n * e^{-mu ln2}) bf16  (= ln n - mu ln2, via the Ln scale)
  ab  = max(w, -w) f16;  w0f = ab + 2*cmw f16
  C_k = C_{k-1} * (Lp/k),  W_k = W_{k-1} * dl    (bf16 TensorTensor)
  I   = C0^T@ab (f16 mm; the missing cmw part of k=0 is column-constant
        and cancels exactly in the pos-neg diff) + sum_k C_k^T@W_k.
Total err ~3.7e-3 (fp16 inputs + bf16 chains) vs the 2e-2 gate.
Inputs DMA as fp16: x blocked [P,9,B] (host-floored at 6.2e-5 so Ln
never sees 0/denormals) and split across both HWDGE queues into two
tiles so Ln starts as soon as the first half lands; (w,n) blocked
[P,2,9,JC].
"""

from contextlib import ExitStack

import numpy as np

import concourse.bass as bass
import concourse.bass_isa as bass_isa
import concourse.tile as tile
from concourse import bacc
from concourse import mybir
from concourse import bass_utils

P = 128
B = 128
N_IN = 1024
N_OUT = 512
NCH = 9                 # i-chunks of 128 (1025 rows padded to 1152)
IPAD = NCH * P
JC = 128                # columns per core
NO = JC // 2            # outputs per core
NCORES = 8
K_TERMS = 3             # series terms k = 0..3

MU = 1.58
LN2 = float(np.log(2.0))
INV_LN2 = 1.0 / LN2
MULN2 = MU * LN2
S_N = float(np.exp(-MULN2))   # Ln scale: ln(n*S_N) = ln n - mu ln2
CB2 = 1.0 / 99.0              # cmw2 = mw/99 (2x cmw; 0.5 lives in C0)
LP_CLAMP = -144.0
X_FLOOR = 6.2e-5              # fp16 min normal; applied in host cast

F32 = mybir.dt.float32
F16 = mybir.dt.float16
BF16 = mybir.dt.bfloat16
AF = mybir.ActivationFunctionType
ALU = mybir.AluOpType

_NC_CACHE = None


def _kernel_body(ctx, tc, xt, wn, y):
    nc = tc.nc

    pool = ctx.enter_context(tc.tile_pool(name="main", bufs=2))
    cpool = ctx.enter_context(tc.tile_pool(name="cpool", bufs=K_TERMS))
    wpool = ctx.enter_context(tc.tile_pool(name="wpool", bufs=K_TERMS))
    psum = ctx.enter_context(tc.tile_pool(name="psum", bufs=2, space="PSUM"))

    # ---- loads on qSP (pre-blocked [p, ...]; one contiguous run per
    # partition); x first: it heads the Ln->Exp critical chain. ----
    xta = pool.tile([P, 5, B], F16, tag="xta")
    nc.sync.dma_start(xta[:], xt.ap()[:, 0:5])
    xtb = pool.tile([P, NCH - 5, B], F16, tag="xtb")
    nc.scalar.dma_start(xtb[:], xt.ap()[:, 5:NCH])
    wnt = pool.tile([P, 2, NCH, JC], F16, tag="wn")
    nc.scalar.dma_start(wnt[:, 1], wn.ap()[:, 1])
    nc.sync.dma_start(wnt[:, 0], wn.ap()[:, 0])

    # ---- ACT: L = ln(2x); dl = ln n - mu ln2 (via scale); C0 = 0.5 e^{mu L}.
    # Both Ln's precede Exp so only two act-table loads happen. ----
    eb = pool.tile([P, 1], F32, tag="eb")
    nc.any.memset(eb[:], -LN2)
    lt = pool.tile([P, NCH, B], F32, tag="lt")
    nc.scalar.activation(lt[:, 0:5], xta[:], AF.Ln, bias=0.0, scale=2.0)
    nc.scalar.activation(lt[:, 5:NCH], xtb[:], AF.Ln, bias=0.0, scale=2.0)
    dl = pool.tile([P, NCH, JC], BF16, tag="dl")
    nc.scalar.activation(dl[:], wnt[:, 1], AF.Ln, bias=0.0, scale=S_N)
    c0 = pool.tile([P, NCH, B], F16, tag="c0")
    nc.scalar.activation(c0[:], lt[:], AF.Exp, bias=eb[:], scale=MU)

    # ---- ab = |w| = max(w, -w) f16 (k=0 rhs, cmw-free) ----
    ngw = pool.tile([P, NCH, JC], F16, tag="ngw")
    nc.vector.tensor_scalar_mul(ngw[:], wnt[:, 0], -1.0)
    ab = pool.tile([P, NCH, JC], F16, tag="ab")
    nc.vector.tensor_tensor(ab[:], wnt[:, 0], ngw[:], op=ALU.max)

    # ---- local max |w| (bias row rides in chunk 8) -> cmw2 = mw/99 ----
    rm = pool.tile([P, 1], F32, tag="rm")
    nc.vector.tensor_reduce(
        rm[:], wnt[:, 0], axis=mybir.AxisListType.XY, op=ALU.max,
        apply_absolute_value=True,
    )
    mwall = pool.tile([P, 1], F32, tag="mwall")
    nc.gpsimd.partition_all_reduce(
        mwall[:], rm[:], channels=P, reduce_op=bass_isa.ReduceOp.max
    )
    cmw = pool.tile([P, 1], F32, tag="cmw")
    nc.vector.tensor_scalar_mul(cmw[:], mwall[:], CB2)

    # ---- Lp = clamp(L/ln2) bf16 + prescaled Lp/k for the C-chain ----
    lp = pool.tile([P, NCH, B], BF16, tag="lp")
    nc.vector.tensor_scalar(lp[:], lt[:], INV_LN2, LP_CLAMP, op0=ALU.mult, op1=ALU.max)
    lpk = {1: lp}
    for k in range(2, K_TERMS + 1):
        t = pool.tile([P, NCH, B], BF16, tag=f"lp{k}")
        nc.vector.tensor_scalar_mul(t[:], lp[:], 1.0 / k)
        lpk[k] = t

    w0f = pool.tile([P, NCH, JC], F16, tag="w0f")
    nc.vector.tensor_scalar(w0f[:], ab[:], 1.0, cmw[:], op0=ALU.mult, op1=ALU.add)

    # ---- PSUM accumulation: k=0 (f16), k=1..3 (bf16 chains).
    # W-link emitted before C-link: the W side is ready earlier. ----
    ps = psum.tile([B, JC], F32, tag="acc")
    for c in range(NCH):
        nc.tensor.matmul(
            ps[:], lhsT=c0[:, c, :], rhs=ab[:, c, :],
            start=(c == 0), stop=False,
        )
    ck, wk = c0, w0f
    for k in range(1, K_TERMS + 1):
        wnew = wpool.tile([P, NCH, JC], BF16, tag="wk")
        nc.vector.tensor_mul(wnew[:], wk[:], dl[:])
        cn = cpool.tile([P, NCH, B], BF16, tag="ck")
        nc.vector.tensor_mul(cn[:], ck[:], lpk[k][:])
        ck, wk = cn, wnew
        for c in range(NCH):
            nc.tensor.matmul(
                ps[:], lhsT=ck[:, c, :], rhs=wk[:, c, :],
                start=False, stop=(k == K_TERMS and c == NCH - 1),
            )

    # ---- y = pos block - neg block (host re-ordered columns) ----
    yp = pool.tile([B, NO], F32, tag="yp")
    nc.scalar.activation(yp[:], ps[:, 0:NO], AF.Copy, bias=0.0, scale=1.0)
    yt = pool.tile([B, NO], F32, tag="yt")
    nc.vector.tensor_sub(yt[:], yp[:], ps[:, NO:JC])
    nc.scalar.dma_start(y.ap(), yt[:])


def build_nc(repeat=1):
    nc = bacc.Bacc(
        "TRN2", target_bir_lowering=False, debug=False, num_devices=NCORES
    )
    xt = nc.dram_tensor("xt", [P, NCH, B], F16, kind="ExternalInput")
    wn = nc.dram_tensor("wn", [P, 2, NCH, JC], F16, kind="ExternalInput")
    y = nc.dram_tensor("y", [B, NO], F32, kind="ExternalOutput")
    with tile.TileContext(nc) as tc:
        with ExitStack() as ctx:
            if repeat == 1:
                _kernel_body(ctx, tc, xt, wn, y)
            else:
                with tc.For_i(0, repeat, 1):
                    _kernel_body(ctx, tc, xt, wn, y)
    nc.compile()
    return nc


def _block(a):
    """[NCH*P, W] row-major -> [P, NCH, W] partition-major contiguous."""
    n, w = a.shape
    return a.reshape(n // P, P, w).transpose(1, 0, 2)


def make_in_maps(x, w_pos, w_neg, b_pos, b_neg, n_devices):
    comb = np.zeros((IPAD, 2 * N_OUT), np.float32)
    comb[:N_IN, 0::2] = w_pos
    comb[:N_IN, 1::2] = w_neg
    comb[N_IN, 0::2] = b_pos
    comb[N_IN, 1::2] = b_neg
    nfull = np.full((IPAD, 2 * N_OUT), 2.0, np.float32)
    nfull[:N_IN + 1] = np.asarray(n_devices, np.float32)
    # inputs with the bias-1 row at i=1024; pad rows floored (w=0 there)
    xfull = np.full((IPAD, B), X_FLOOR, np.float32)
    xfull[:N_IN] = np.asarray(x, np.float32).T
    xfull[N_IN] = 1.0
    xq = np.maximum(xfull.astype(np.float16), np.float16(X_FLOOR))
    xb = np.ascontiguousarray(_block(xq))            # [P, NCH, B]
    # within-core column order: 64 pos then 64 neg
    perm = np.r_[np.arange(0, JC, 2), np.arange(1, JC, 2)]
    in_maps = []
    for core in range(NCORES):
        js = slice(JC * core, JC * (core + 1))
        wc = comb[:, js][:, perm]
        ncr = nfull[:, js][:, perm]
        wnb = np.stack([_block(wc), _block(ncr)], axis=1).astype(np.float16)
        in_maps.append({"xt": xb, "wn": np.ascontiguousarray(wnb)})
    return in_maps


def gather(results):
    return np.concatenate(
        [np.asarray(results[c]["y"], np.float32) for c in range(NCORES)], axis=1
    )


def _get_nc():
    global _NC_CACHE
    if _NC_CACHE is None:
        _NC_CACHE = build_nc()
    return _NC_CACHE


def kernel(x, w_pos, w_neg, b_pos, b_neg, n_devices):
    in_maps = make_in_maps(x, w_pos, w_neg, b_pos, b_neg, n_devices)
    res = bass_utils.run_bass_kernel_spmd(
        _get_nc(), in_maps, core_ids=list(range(NCORES))
    )
    return gather(res.results)
